# revision 107
# baseline (speedup 1.0000x reference)
"""Distributed GQA attention block (B=2, S=2048, D=2048, H=16, KV=4, HD=128,
RoPE, causal) on 8 Trainium2 NeuronCores.

Sharding: tensor-parallel over heads. Core i computes q-heads {2i, 2i+1} and
kv-head i//2. Each core produces a partial output projection (its heads'
columns of wo); the host sums the 8 partials.

Q/K/V projections run as fp8e4 DoubleRow matmuls with both operands split
into hi+lo fp8 components; the three significant cross products
(hi*hi + hi*lo + lo*hi) reproduce bf16-or-better accuracy at 0.75x the
bf16 PE cycle count. Weights are pre-scaled by a power of two on the host
(fp8 dynamic-range centering) and the inverse scale is folded into the
PSUM evacuation. V is produced directly in natural [tk, hd] layout with x
as the stationary operand. Attention (scores, softmax, PV, output
projection) runs in bf16 with the transposed-strip dataflow: S^T = k^T.T q^T
per kv chunk, exp written directly into P^T strips, O accumulated via
[P^T | ones]-style extended PV matmuls (ones column of V accumulates the
softmax denominators), PE-transposed to O^T, and projected y^T = wo^T.T O^T.
"""

import math
from collections import deque
from functools import partial

import numpy as np
import ml_dtypes

B, S, D = 2, 2048, 2048
H, KV, HD = 16, 4, 128
NCORES = 8
HPC = H // NCORES  # q heads per core
THETA = 10000.0

ND = D // 128  # 128-deep contraction chunks
NT = S // 512  # 512-wide t-blocks per batch
NI = S // 128  # 128-wide tq/tk chunks per batch

KQ = 9  # wq (with 1/sqrt(HD) folded) fp8 pre-scale: 2^9
KK = 6  # wk/wv fp8 pre-scale: 2^6
KW = 6  # wo fp8 pre-scale: 2^6
KO = 4  # on-device O^T fp8 pre-scale: 2^4 (folded into the PV normalize)

_BUILD_CACHE = {}


def _split_multi_waits(nc, max_waits=1):
    """This walrus build rejects >1 sync wait per instruction. Move extra
    semaphore waits onto no-ops inserted before the instruction on the same
    engine."""
    import concourse.mybir as mybir

    n_split = 0
    for f in nc.m.functions:
        for bb in f.blocks:
            insts = bb.instructions
            i = 0
            while i < len(insts):
                inst = insts[i]
                si = getattr(inst, "sync_info", None)
                if si is not None and si.on_wait and len(si.on_wait) > max_waits:
                    waits = list(si.on_wait)
                    extra, keep = waits[:-max_waits], waits[-max_waits:]
                    si.on_wait = keep
                    inst.sync_info = si
                    for j, w in enumerate(extra):
                        noop = mybir.InstNoOp(
                            name=f"{inst.name}-wsplit{j}",
                            sync_info=mybir.SyncInfo(on_wait=[w], on_update=[]),
                            bass_nofuse=True,
                            engine=inst.engine,
                        )
                        try:
                            nc.register_instruction(noop, overwrite=True)
                        except Exception:
                            pass
                        insts.insert(i + j, noop)
                        n_split += 1
                    i += len(extra)
                i += 1
    return n_split


def _build():
    import concourse.bass as bass
    import concourse.mybir as mybir
    from concourse import tile
    from concourse.masks import make_identity, make_upper_triangular

    BF16, F32, F8 = mybir.dt.bfloat16, mybir.dt.float32, mybir.dt.float8e4
    MULT, ADD = mybir.AluOpType.mult, mybir.AluOpType.add
    EXP = mybir.ActivationFunctionType.Exp
    DR = mybir.MatmulPerfMode.DoubleRow

    nc = bass.Bass()
    # weights arrive pre-swizzled into the SBUF layout [128, ND*M] (2KB+
    # contiguous rows: full-rate DMA descriptors, no rearrange cost)
    xh_e = nc.declare_dram_parameter("xhiT", [B, D, S], F8, isOutput=False)
    xl_e = nc.declare_dram_parameter("xloT", [B, D, S], F8, isOutput=False)
    wq_h_e = nc.declare_dram_parameter("wqhiT", [128, ND * HPC * HD], F8, isOutput=False)
    wq_l_e = nc.declare_dram_parameter("wqloT", [128, ND * HPC * HD], F8, isOutput=False)
    wk_h_e = nc.declare_dram_parameter("wkhiT", [128, ND * HD], F8, isOutput=False)
    wk_l_e = nc.declare_dram_parameter("wkloT", [128, ND * HD], F8, isOutput=False)
    wv_h_e = nc.declare_dram_parameter("wvhiT", [128, ND * HD], F8, isOutput=False)
    wv_l_e = nc.declare_dram_parameter("wvloT", [128, ND * HD], F8, isOutput=False)
    wo_h_e = nc.declare_dram_parameter("wohiT", [128, HPC * D], F8, isOutput=False)
    wo_l_e = nc.declare_dram_parameter("woloT", [128, HPC * D], F8, isOutput=False)
    cos_e = nc.declare_dram_parameter("cosT", [HD // 2, S], BF16, isOutput=False)
    sin_e = nc.declare_dram_parameter("sinT", [HD // 2, S], BF16, isOutput=False)
    yT_e = nc.declare_dram_parameter("yT", [D, B * S], BF16, isOutput=True)

    with tile.TileContext(nc) as tc:
        with (
            tc.tile_pool(name="const", bufs=1) as cpool,
            tc.tile_pool(name="w", bufs=1) as wpool,
            tc.tile_pool(name="x", bufs=1) as xpool,
            tc.tile_pool(name="act", bufs=1) as apool,
            tc.tile_pool(name="tmp", bufs=3) as tpool,
            tc.tile_pool(name="psA", bufs=4, space="PSUM") as psA,
            tc.tile_pool(name="psS", bufs=2, space="PSUM") as psS,
            tc.tile_pool(name="psB", bufs=2, space="PSUM") as psB,
        ):
            # ---- weight / table loads (nc.sync HWDGE), interleaved with the
            # first x tiles so the PE can start within ~4us.
            wk_h = wpool.tile([128, ND, HD], F8, tag="wkh", name="wkh")
            wk_l = wpool.tile([128, ND, HD], F8, tag="wkl", name="wkl")
            wq_h = wpool.tile([128, ND, HPC * HD], F8, tag="wqh", name="wqh")
            wq_l = wpool.tile([128, ND, HPC * HD], F8, tag="wql", name="wql")
            wv_h = wpool.tile([128, ND, HD], F8, tag="wvh", name="wvh")
            wv_l = wpool.tile([128, ND, HD], F8, tag="wvl", name="wvl")

            def ld_w(t, e, halves=1):
                # halves=2 splits the load so matmuls gated on the first
                # chunks can start after half the bytes land
                ea = e.rearrange("p (d o) -> p d o", d=ND)
                for i in range(halves):
                    sl = slice(i * ND // halves, (i + 1) * ND // halves)
                    nc.sync.dma_start(t[:, sl, :], ea[:, sl, :])

            xtiles = {}  # (b, hl, tb) -> [4 piece tiles]

            def ld_x(b, tb, hl):
                e = xh_e if hl == 0 else xl_e
                ts = []
                for pc in range(4):
                    t = xpool.tile(
                        [128, 4, 512], F8, tag=f"x{hl}{tb}{pc}",
                        name=f"x{hl}{tb}{pc}",
                    )
                    nc.sync.dma_start(
                        t[:],
                        e[
                            b,
                            pc * 512 : (pc + 1) * 512,
                            tb * 512 : (tb + 1) * 512,
                        ].rearrange("(d p) s -> p d s", p=128),
                    )
                    ts.append(t)
                xtiles[(b, hl, tb)] = ts

            # first k/q matmuls gate on these: thin, ordered loads. V runs as
            # a second pass so its weights load late; x tiles stream tb-major.
            # load order mirrors the A-first projection schedule: hi x tiles
            # and hi weights first, then the lo corrections, then tables/V/O.
            ld_x(0, 0, 0)
            ld_w(wk_h, wk_h_e, halves=2)
            ld_w(wq_h, wq_h_e, halves=2)
            ld_x(0, 1, 0)
            ld_x(0, 0, 1)
            ld_w(wk_l, wk_l_e)
            ld_w(wq_l, wq_l_e)
            cosT = cpool.tile([HD, S], BF16, tag="cos", name="cos")
            nc.sync.dma_start(cosT[0:64, :], cos_e[:, :])
            sinT = cpool.tile([HD, S], BF16, tag="sin", name="sin")
            nc.sync.dma_start(sinT[0:64, :], sin_e[:, :])
            nc.vector.tensor_copy(cosT[64:128, :], cosT[0:64, :])
            nc.vector.tensor_scalar_mul(sinT[64:128, :], sinT[0:64, :], -1.0)
            ld_x(0, 1, 1)
            ld_x(0, 2, 0)
            ld_x(0, 2, 1)
            ld_x(0, 3, 0)
            ld_x(0, 3, 1)
            ld_w(wv_h, wv_h_e)
            ld_w(wv_l, wv_l_e)
            wo_h = wpool.tile([128, HPC, D], F8, tag="woh", name="woh")
            nc.sync.dma_start(wo_h[:], wo_h_e.rearrange("p (c o) -> p c o", c=HPC))
            wo_l = wpool.tile([128, HPC, D], F8, tag="wol", name="wol")
            nc.sync.dma_start(wo_l[:], wo_l_e.rearrange("p (c o) -> p c o", c=HPC))

            ident = cpool.tile([128, 128], BF16, tag="ident", name="ident")
            make_identity(nc, ident[:])
            triu = cpool.tile([128, 128], BF16, tag="triu", name="triu")
            make_upper_triangular(nc, triu[:], val=1.0, diag=True)

            def rope(dst, acc, dsl, sl, scale, act_evac=True):
                # dsl: destination column slice in dst; sl: token slice for
                # the cos/sin tables (differ for the per-tb kTr tiles)
                ev = tpool.tile([128, 512], BF16, tag="ropee", name="ropee", bufs=2)
                if act_evac:
                    nc.scalar.mul(ev[:], acc[:], scale)
                else:
                    # DVE evac variant for fillers inside exp-bound windows
                    nc.vector.tensor_scalar_mul(ev[:], acc[:], scale)
                sw = tpool.tile([128, 512], BF16, tag="ropesw", name="ropesw", bufs=1)
                nc.vector.tensor_copy(sw[0:64, :], ev[64:128, :])
                nc.vector.tensor_copy(sw[64:128, :], ev[0:64, :])
                nc.vector.tensor_tensor(sw[:], sw[:], sinT[:, sl], op=MULT)
                nc.vector.tensor_tensor(ev[:], ev[:], cosT[:, sl], op=MULT)
                nc.vector.tensor_tensor(dst[:, dsl], ev[:], sw[:], op=ADD)

            # ---------- cross-batch pipelined schedule ----------
            # Batch b+1's k/q projection units run as PE fillers inside batch
            # b's attention loop (whose early strips are exp/ACT-bound), and
            # batch b's final oproj block runs as fillers in batch b+1's early
            # strips. qTr/kTr and x tiles are double-buffered to decouple the
            # batches; everything else reuses the same buffers (Tile's
            # region-granular deps keep it correct).
            yscale = 2.0 ** -(KW + KO)

            def dr_part(acc, w_h, w_l, csl, b, tb, part):
                xh = xtiles[(b, 0, tb)]
                xl = xtiles[(b, 1, tb)]
                ops = [(w_h, xh)] if part == 0 else [(w_h, xl), (w_l, xh)]
                n = 0 if part == 0 else ND // 2
                for wt, xt in ops:
                    for p in range(ND // 2):
                        lp = p % 2
                        nc.tensor.matmul(
                            acc[:],
                            wt[:, 2 * p : 2 * p + 2, csl],
                            xt[p // 2][:, 2 * lp : 2 * lp + 2, :],
                            start=(n == 0),
                            stop=(n == 3 * (ND // 2) - 1),
                            perf_mode=DR,
                        )
                        n += 1

            def alloc_qk():
                # q/k live in per-tb tiles (scores chunks are aligned to
                # absolute 512-col boundaries, so each chunk stays within one
                # tb tile). Single-buffered is then deadlock-free: the batch
                # b+1 rope for tb is emitted after every batch-b scores strip
                # that reads those columns (strips j < 4*tb+4), so its
                # whole-tile WAR wait only ever points backwards.
                qTr = [
                    [
                        apool.tile([HD, 512], BF16, tag=f"q{h}{tb}",
                                   name=f"q{h}{tb}")
                        for tb in range(NT)
                    ]
                    for h in range(HPC)
                ]
                kTr = [
                    apool.tile([HD, 512], BF16, tag=f"k{tb}", name=f"k{tb}")
                    for tb in range(NT)
                ]
                return qTr, kTr

            def kqproj_units(qTr, kTr):
                units = []
                for tb in range(NT):
                    units.append(
                        (wk_h, wk_l, slice(0, HD), tb, kTr[tb],
                         slice(0, 512), 2.0**-KK)
                    )
                    for h in range(HPC):
                        units.append(
                            (wq_h, wq_l, slice(h * HD, (h + 1) * HD), tb,
                             qTr[h][tb], slice(0, 512), 2.0**-KQ)
                        )
                return units

            def emit_unit(u, b, act_evac=True):
                w_h, w_l, csl, tb, dst, dsl, sc = u
                acc = psA.tile([128, 512], mybir.dt.float32, tag="acc",
                               name="acc")
                dr_part(acc, w_h, w_l, csl, b, tb, 0)
                dr_part(acc, w_h, w_l, csl, b, tb, 1)
                rope(dst, acc, dsl, slice(tb * 512, (tb + 1) * 512), sc,
                     act_evac)

            def kqproj_startup(b, qTr, kTr):
                # Startup k/q projection: per tb-pair, the six hi*hi product
                # chains first (they only need the hi x/w tiles, which arrive
                # first), then the hi*lo / lo*hi corrections + evacuations.
                # Tracks the DMA arrival order so the PE starts within ~4us.
                all_units = kqproj_units(qTr, kTr)
                for tbp in range(NT // 2):
                    units = [
                        all_units[i]
                        for i in range(3 * 2 * tbp, 3 * 2 * tbp + 6)
                    ]
                    if tbp == NT // 2 - 1:
                        # q ropes first on the final pair: the first scores
                        # strips consume qTr, k chunk 0 is already resident
                        units = units[1:3] + [units[0]] + units[4:6] + [units[3]]
                    accs = []
                    for w_h, w_l, csl, tb, dst, dsl, sc in units:
                        acc = psA.tile(
                            [128, 512], mybir.dt.float32, tag="acc", name="acc"
                        )
                        dr_part(acc, w_h, w_l, csl, b, tb, 0)
                        accs.append(acc)
                    for acc, (w_h, w_l, csl, tb, dst, dsl, sc) in zip(
                        accs, units
                    ):
                        dr_part(acc, w_h, w_l, csl, b, tb, 1)
                        rope(dst, acc, dsl, slice(tb * 512, (tb + 1) * 512),
                             sc)

            def oproj_quad(b, oth, otl, cg, q, act_ok=True):
                # Four dc-blocks in two [128, 2, 512] staging tiles + two
                # strided DMAs. Quads are spread one-per-strip so the PSUM
                # ring never sees a 16-tile demand burst. act_ok=False keeps
                # every evac on DVE (for exp-saturated windows).
                tsl = slice(cg * 512, (cg + 1) * 512)
                for half in range(2):
                    yrq = tpool.tile([128, 2, 512], BF16, tag="yrq",
                                     name="yrq", bufs=3)
                    for i in range(2):
                        dc = q * 4 + half * 2 + i
                        dsl = slice(dc * 128, (dc + 1) * 128)
                        yps = psA.tile([128, 512], mybir.dt.float32,
                                       tag="acc", name="yps")
                        for n, (wt, ot) in enumerate(
                            ((wo_h, oth), (wo_h, otl), (wo_l, oth))
                        ):
                            nc.tensor.matmul(
                                yps[:],
                                wt[:, :, dsl],
                                ot[:, :, tsl],
                                start=(n == 0),
                                stop=(n == 2),
                                perf_mode=DR,
                            )
                        if act_ok and half * 2 + i == 1:
                            nc.scalar.mul(yrq[:, i, :], yps[:], yscale)
                        else:
                            nc.vector.tensor_scalar_mul(yrq[:, i, :], yps[:],
                                                        yscale)
                    nc.gpsimd.dma_start(
                        yT_e[q * 512 + half * 256 : q * 512 + half * 256 + 256,
                             b * S + cg * 512 : b * S + (cg + 1) * 512]
                        .rearrange("(d p) s -> p d s", p=128),
                        yrq[:],
                    )

            def oproj_tail(b, oth, otl, c0, groups):
                # Fine-grained tail for the last batch's final token blocks:
                # a 256-wide column slab, split into quarter-height DMAs
                # (256-col bf16 rows = 512B descriptors: still full DMA rate)
                # so the staging tag stays a small double buffer.
                tsl = slice(c0, c0 + 256)
                for g in groups:
                    # staging reuses the yrq tag: [128, 2, 512] has the same
                    # per-partition linear layout as the [128, 4, 256] DMA view
                    yt = tpool.tile([128, 2, 512], BF16, tag="yrq",
                                    name="yrq", bufs=3)
                    for i in range(4):
                        dc = g * 4 + i
                        dsl = slice(dc * 128, (dc + 1) * 128)
                        ysl = slice((i % 2) * 256, (i % 2) * 256 + 256)
                        yps = psA.tile([128, 512], mybir.dt.float32,
                                       tag="acc", name="yps")
                        for n, (wt, ot) in enumerate(
                            ((wo_h, oth), (wo_h, otl), (wo_l, oth))
                        ):
                            nc.tensor.matmul(
                                yps[:, 0:256],
                                wt[:, :, dsl],
                                ot[:, :, tsl],
                                start=(n == 0),
                                stop=(n == 2),
                                perf_mode=DR,
                            )
                        if i % 2 == 1:
                            nc.scalar.mul(yt[:, i // 2, ysl], yps[:, 0:256],
                                          yscale)
                        else:
                            nc.vector.tensor_scalar_mul(
                                yt[:, i // 2, ysl], yps[:, 0:256], yscale
                            )
                    # HWDGE: the sync queue is idle by the tail, and its
                    # per-DMA latency is far below the engine SWDGE path —
                    # these DMAs gate the final drain
                    nc.sync.dma_start(
                        yT_e[g * 512 : (g + 1) * 512,
                             b * S + c0 : b * S + c0 + 256]
                        .rearrange("(d p) s -> p d s", p=128),
                        yt[:],
                    )

            def attention(b, qTr, kTr, fillers):
                """Attention for batch b; `fillers` is a deque of (slot, fn)
                PE-filler closures emitted once the loop index reaches slot.
                Returns filler closures for batch b+1 (this batch's final
                oproj block), or [] for the last batch (handled via the
                fine-grained tail)."""
                vnat = [
                    apool.tile([128, HD + 1], BF16, tag=f"vn{j}", name=f"vn{j}")
                    for j in range(NI)
                ]

                def vproj(j):
                    # V natural: x stationary, wv moving, per 128-wide
                    # t-chunk. Runs inside the attention loop (chunk j is
                    # first needed by pv(c=j) one block later) to fill the PE
                    # during the exp-heavy early strips.
                    tb, i = divmod(j, 4)
                    vacc = psA.tile(
                        [128, HD], mybir.dt.float32, tag="acc", name="vacc"
                    )
                    xh = xtiles[(b, 0, tb)]
                    xl = xtiles[(b, 1, tb)]
                    tsl = slice(i * 128, (i + 1) * 128)
                    ops = [(xh, wv_h), (xl, wv_h), (xh, wv_l)]
                    n = 0
                    for xt, wt in ops:
                        for p in range(ND // 2):
                            lp = p % 2
                            nc.tensor.matmul(
                                vacc[:],
                                xt[p // 2][:, 2 * lp : 2 * lp + 2, tsl],
                                wt[:, 2 * p : 2 * p + 2, :],
                                start=(n == 0),
                                stop=(n == 3 * (ND // 2) - 1),
                                perf_mode=DR,
                            )
                            n += 1
                    # DVE evac: keeps ACT free for the exp stream. The ones
                    # column is pre-scaled by 2^-KO so the PV-normalize
                    # reciprocal comes out as 2^KO/denom (the fp8 O^T
                    # pre-scale), letting the normalize run on ACT as a
                    # single scaled copy.
                    nc.vector.tensor_scalar_mul(vnat[j][:, 0:HD], vacc[:],
                                                2.0**-KK)
                    nc.gpsimd.memset(vnat[j][:, HD : HD + 1], 2.0**-KO)

                strips = {h: [] for h in range(HPC)}
                for h in range(HPC):
                    for j in range(NI):
                        strips[h].append(
                            apool.tile(
                                [128, S - j * 128],
                                BF16,
                                tag=f"pt{h}{j}",
                                name=f"pt{h}{j}",
                            )
                        )
                # O^T in fp8 hi+lo (oproj runs as 3-product DoubleRow):
                # [:, h, :] holds head h's 128 dims, so [:, :, tsl] is the
                # natural DR moving layout pairing both heads' chunks.
                # Double-buffered: batch b's oproj quads spread as fillers
                # through batch b+1's attention window.
                oth = apool.tile([128, HPC, S], F8, tag="oth", name="oth",
                                 bufs=2)
                otl = apool.tile([128, HPC, S], F8, tag="otl", name="otl",
                                 bufs=2)

                def scores(j, h):
                    # chunks aligned to absolute 512-col boundaries: same
                    # chunk count as naive splitting, but each chunk reads a
                    # single per-tb qTr tile
                    pts = strips[h][j]
                    a = j * 128
                    while a < S:
                        w = min(512 - a % 512, S - a)
                        sps = psS.tile([128, 512], mybir.dt.float32,
                                       tag="sps", name="s")
                        nc.tensor.matmul(
                            sps[:, :w],
                            kTr[j // 4][:, (j % 4) * 128 : (j % 4 + 1) * 128],
                            qTr[h][a // 512][:, a % 512 : a % 512 + w],
                            start=True,
                            stop=True,
                        )
                        c0 = a - j * 128
                        nc.scalar.activation(pts[:, c0 : c0 + w], sps[:, :w],
                                             EXP)
                        if c0 == 0:
                            # causal mask on Pool: DVE is the loaded engine
                            # in the attention windows, Pool is nearly idle
                            nc.gpsimd.tensor_tensor(
                                pts[:, 0:128], pts[:, 0:128], triu[:], op=MULT
                            )
                        a += w

                # pv stage 1: PV matmuls + kick the DVE normalize; stage 2
                # (one strip later): PE transpose + split into fp8 O^T hi/lo,
                # so the PE never waits on the DVE chain.
                pv_osb = {}

                def pv_mm(c, h):
                    oext = psA.tile([128, 512], mybir.dt.float32, tag="acc",
                                    name="oext")
                    for j in range(c + 1):
                        nc.tensor.matmul(
                            oext[:, 0 : HD + 1],
                            strips[h][j][:, (c - j) * 128 : (c - j + 1) * 128],
                            vnat[j][:],
                            start=(j == 0),
                            stop=(j == c),
                        )
                    osb = tpool.tile([128, HD], BF16, tag=f"onat{h}",
                                     name="onat", bufs=2)
                    rcol = tpool.tile([128, 1], mybir.dt.float32, tag=f"rc{h}",
                                      name="rcol", bufs=2)
                    nc.vector.reciprocal(rcol[:], oext[:, HD : HD + 1])
                    # normalize (rcol = 2^KO/denom via the scaled ones
                    # column) as a scaled copy on ACT for the mid strips:
                    # keeps the DVE queue, which gates the downstream
                    # transposes, short. Early strips (ACT exp-bound) and the
                    # last strips (ACT gates the tail evacs) stay on DVE.
                    if 6 <= c:
                        nc.scalar.activation(
                            osb[:], oext[:, 0:HD],
                            mybir.ActivationFunctionType.Copy, scale=rcol[:]
                        )
                    else:
                        nc.vector.tensor_scalar(
                            osb[:], oext[:, 0:HD], rcol[:], None, op0=MULT
                        )
                    pv_osb[(c, h)] = osb

                def pv_tp(c, h):
                    tp = psB.tile([128, 128], BF16, tag="pt", name="pt")
                    nc.tensor.transpose(tp[:], pv_osb.pop((c, h))[:], ident[:])
                    csl = slice(c * 128, (c + 1) * 128)
                    nc.vector.tensor_copy(oth[:, h, csl], tp[:])
                    nc.vector.tensor_tensor(
                        otl[:, h, csl], tp[:], oth[:, h, csl],
                        op=mybir.AluOpType.subtract,
                    )

                # tb0's four v-chunks up front: this batch's x tb0 is then
                # dead, so the next batch's tb0 load (WAR-ordered) starts at
                # the top of this window. The rest run two-per-strip — the
                # early strips are exp-bound and need the PE filler, and x
                # tbs die sooner, pulling the next batch's loads forward.
                for jj in range(4):
                    vproj(jj)
                if b + 1 < B:
                    ld_x(b + 1, 0, 0)
                    ld_x(b + 1, 0, 1)
                last = b + 1 >= B
                quads = []
                vnext = 4
                for j in range(NI + 2):
                    if j < NI:
                        for h in range(HPC):
                            scores(j, h)
                    for _ in range(2):
                        if vnext < NI:
                            vproj(vnext)
                            if vnext % 4 == 3 and b + 1 < B:
                                ld_x(b + 1, vnext // 4, 0)
                                ld_x(b + 1, vnext // 4, 1)
                            vnext += 1
                    if 1 <= j <= NI:
                        for h in range(HPC):
                            pv_mm(j - 1, h)
                    if j >= 2:
                        c2 = j - 2
                        for h in range(HPC):
                            pv_tp(c2, h)
                        if c2 % 4 == 3 and c2 // 4 < NT - 1:
                            quads.extend((c2 // 4, q) for q in range(4))
                        if quads:
                            # the last batch's window is exp-saturated with
                            # no next-batch fillers: keep its quad evacs on
                            # DVE
                            oproj_quad(b, oth, otl, *quads.pop(0),
                                       act_ok=True)
                        if last:
                            # fine-grained tail for the final 512 tokens as
                            # two strip-pair slabs, so only the last ~1MB of
                            # output DMA trails the final transposes
                            if c2 == NI - 3:
                                oproj_tail(b, oth, otl, S - 512, range(0, 2))
                            elif c2 == NI - 2:
                                oproj_tail(b, oth, otl, S - 512, range(2, 4))
                            elif c2 == NI - 1:
                                oproj_tail(b, oth, otl, S - 256, range(4))
                    while fillers and fillers[0][0] <= j:
                        fillers.popleft()[1]()
                while fillers:
                    fillers.popleft()[1]()
                if last:
                    return []
                # the final block cg=3 goes to the next batch's early window
                # (oth/otl are double-buffered, so these reads never collide
                # with batch b+1's pv_tp writes)
                cg = NT - 1
                return [
                    (q, partial(oproj_quad, b, oth, otl, cg, q, False))
                    for q in range(4)
                ]

            qk = [None] * B
            qk[0] = alloc_qk()
            kqproj_startup(0, *qk[0])
            fillers = deque()
            for b in range(B):
                if b + 1 < B:
                    qk[b + 1] = alloc_qk()
                    units = kqproj_units(*qk[b + 1])
                    for i, u in enumerate(units):
                        # one unit per strip from slot 3: with the 2x vproj
                        # pacing, x tb lands ~2.5 strips before its units;
                        # early slots evacuate the rope via DVE: ACT is
                        # exp-saturated in the wide strips
                        fillers.append(
                            (3 + i, partial(emit_unit, u, b + 1, i >= 4))
                        )
                nxt = attention(b, *qk[b], fillers)
                fillers = deque(nxt)

    _split_multi_waits(nc)
    nc.finalize()
    return nc


def _get_nc():
    if "nc" not in _BUILD_CACHE:
        _BUILD_CACHE["nc"] = _build()
    return _BUILD_CACHE["nc"]


def _prep_inputs(x, wq, wk, wv, wo):
    """Host-side shard prep: per-core transposed fp8 hi/lo weight splits and
    shared fp8 hi/lo x^T."""
    bf16 = ml_dtypes.bfloat16
    f8 = ml_dtypes.float8_e4m3

    xT = np.ascontiguousarray(np.transpose(x, (0, 2, 1)))
    xhi = xT.astype(f8)
    xlo = (xT - xhi.astype(np.float32)).astype(f8)

    # RoPE tables in [hd, s] layout; emb = concat([ang, ang]).
    inv_freq = 1.0 / (THETA ** (np.arange(0, HD, 2, dtype=np.float32) / HD))
    ang = np.arange(S, dtype=np.float32)[:, None] * inv_freq[None, :]  # [S, HD/2]
    cosT = np.cos(ang).T.astype(bf16)  # [HD/2, S]; device mirrors to 64..127
    sinT = (-np.sin(ang).T).astype(bf16)  # negated rows 0..63; device flips sign

    scale = 1.0 / math.sqrt(HD)

    def split(a, k):
        # a: [D, M] transposed weight; returns fp8 hi/lo pre-swizzled into
        # the device SBUF layout [128, ND*M] (partition p holds row d*128+p)
        s = np.ascontiguousarray(a * 2.0**k).astype(np.float32)
        hi = s.astype(f8)
        lo = (s - hi.astype(np.float32)).astype(f8)

        def swz(w):
            m = w.shape[1]
            return np.ascontiguousarray(
                w.reshape(ND, 128, m).transpose(1, 0, 2).reshape(128, ND * m)
            )

        return swz(hi), swz(lo)

    in_maps = []
    for c in range(NCORES):
        h0 = c * HPC
        g = (c * HPC) // (H // KV)
        wq_hi, wq_lo = split((wq[h0 * HD : (h0 + HPC) * HD, :] * scale).T, KQ)
        wk_hi, wk_lo = split(wk[g * HD : (g + 1) * HD, :].T, KK)
        wv_hi, wv_lo = split(wv[g * HD : (g + 1) * HD, :].T, KK)
        # wo fp8 hi/lo in [128, HPC*D] (partition p, chunk c -> row c*128+p)
        wo_c = wo[:, h0 * HD : (h0 + HPC) * HD].T  # [HPC*HD, D]
        ws = np.ascontiguousarray(wo_c * 2.0**KW).astype(np.float32)
        wo_hi8 = ws.astype(f8)
        wo_lo8 = (ws - wo_hi8.astype(np.float32)).astype(f8)

        def swz_o(w):
            return np.ascontiguousarray(
                w.reshape(HPC, 128, D).transpose(1, 0, 2).reshape(128, HPC * D)
            )
        in_maps.append(
            {
                "xhiT": xhi,
                "xloT": xlo,
                "wqhiT": wq_hi,
                "wqloT": wq_lo,
                "wkhiT": wk_hi,
                "wkloT": wk_lo,
                "wvhiT": wv_hi,
                "wvloT": wv_lo,
                "wohiT": swz_o(wo_hi8),
                "woloT": swz_o(wo_lo8),
                "cosT": cosT,
                "sinT": sinT,
            }
        )
    return in_maps


def _gather(results):
    acc = np.zeros((D, B * S), np.float32)
    for r in results:
        acc += r["yT"].astype(np.float32)
    return np.ascontiguousarray(acc.T).reshape(B, S, D)


def kernel(x, wq, wk, wv, wo):
    from concourse.bass_utils import run_bass_kernel_spmd

    # Coerce to host numpy: device-array inputs must not trigger on-device
    # host math in _prep_inputs.
    x = np.asarray(x, dtype=np.float32)
    wq = np.asarray(wq, dtype=np.float32)
    wk = np.asarray(wk, dtype=np.float32)
    wv = np.asarray(wv, dtype=np.float32)
    wo = np.asarray(wo, dtype=np.float32)

    nc = _get_nc()
    in_maps = _prep_inputs(x, wq, wk, wv, wo)
    res = run_bass_kernel_spmd(nc, in_maps, core_ids=list(range(NCORES)))
    return _gather(res.results)



# revision 108
# speedup vs baseline: 1.0164x; 1.0164x over previous
"""Distributed GQA attention block (B=2, S=2048, D=2048, H=16, KV=4, HD=128,
RoPE, causal) on 8 Trainium2 NeuronCores.

Sharding: tensor-parallel over heads. Core i computes q-heads {2i, 2i+1} and
kv-head i//2. Each core produces a partial output projection (its heads'
columns of wo); the host sums the 8 partials.

Q/K/V projections run as fp8e4 DoubleRow matmuls with both operands split
into hi+lo fp8 components; the three significant cross products
(hi*hi + hi*lo + lo*hi) reproduce bf16-or-better accuracy at 0.75x the
bf16 PE cycle count. Weights are pre-scaled by a power of two on the host
(fp8 dynamic-range centering) and the inverse scale is folded into the
PSUM evacuation. V is produced directly in natural [tk, hd] layout with x
as the stationary operand. Attention (scores, softmax, PV, output
projection) runs in bf16 with the transposed-strip dataflow: S^T = k^T.T q^T
per kv chunk, exp written directly into P^T strips, O accumulated via
[P^T | ones]-style extended PV matmuls (ones column of V accumulates the
softmax denominators), PE-transposed to O^T, and projected y^T = wo^T.T O^T.
"""

import math
from collections import deque
from functools import partial

import numpy as np
import ml_dtypes

B, S, D = 2, 2048, 2048
H, KV, HD = 16, 4, 128
NCORES = 8
HPC = H // NCORES  # q heads per core
THETA = 10000.0

ND = D // 128  # 128-deep contraction chunks
NT = S // 512  # 512-wide t-blocks per batch
NI = S // 128  # 128-wide tq/tk chunks per batch

KQ = 9  # wq (with 1/sqrt(HD) folded) fp8 pre-scale: 2^9
KK = 6  # wk/wv fp8 pre-scale: 2^6
KW = 6  # wo fp8 pre-scale: 2^6
KO = 4  # on-device O^T fp8 pre-scale: 2^4 (folded into the PV normalize)

_BUILD_CACHE = {}


def _split_multi_waits(nc, max_waits=1):
    """This walrus build rejects >1 sync wait per instruction. Move extra
    semaphore waits onto no-ops inserted before the instruction on the same
    engine."""
    import concourse.mybir as mybir

    n_split = 0
    for f in nc.m.functions:
        for bb in f.blocks:
            insts = bb.instructions
            i = 0
            while i < len(insts):
                inst = insts[i]
                si = getattr(inst, "sync_info", None)
                if si is not None and si.on_wait and len(si.on_wait) > max_waits:
                    waits = list(si.on_wait)
                    extra, keep = waits[:-max_waits], waits[-max_waits:]
                    si.on_wait = keep
                    inst.sync_info = si
                    for j, w in enumerate(extra):
                        noop = mybir.InstNoOp(
                            name=f"{inst.name}-wsplit{j}",
                            sync_info=mybir.SyncInfo(on_wait=[w], on_update=[]),
                            bass_nofuse=True,
                            engine=inst.engine,
                        )
                        try:
                            nc.register_instruction(noop, overwrite=True)
                        except Exception:
                            pass
                        insts.insert(i + j, noop)
                        n_split += 1
                    i += len(extra)
                i += 1
    return n_split


def _build():
    import concourse.bass as bass
    import concourse.mybir as mybir
    from concourse import tile
    from concourse.masks import make_identity, make_upper_triangular

    BF16, F32, F8 = mybir.dt.bfloat16, mybir.dt.float32, mybir.dt.float8e4
    MULT, ADD = mybir.AluOpType.mult, mybir.AluOpType.add
    EXP = mybir.ActivationFunctionType.Exp
    DR = mybir.MatmulPerfMode.DoubleRow

    nc = bass.Bass()
    # weights arrive pre-swizzled into the SBUF layout [128, ND*M] (2KB+
    # contiguous rows: full-rate DMA descriptors, no rearrange cost)
    xh_e = nc.declare_dram_parameter("xhiT", [B, D, S], F8, isOutput=False)
    xl_e = nc.declare_dram_parameter("xloT", [B, D, S], F8, isOutput=False)
    wq_h_e = nc.declare_dram_parameter("wqhiT", [128, ND * HPC * HD], F8, isOutput=False)
    wq_l_e = nc.declare_dram_parameter("wqloT", [128, ND * HPC * HD], F8, isOutput=False)
    wk_h_e = nc.declare_dram_parameter("wkhiT", [128, ND * HD], F8, isOutput=False)
    wk_l_e = nc.declare_dram_parameter("wkloT", [128, ND * HD], F8, isOutput=False)
    wv_h_e = nc.declare_dram_parameter("wvhiT", [128, ND * HD], F8, isOutput=False)
    wv_l_e = nc.declare_dram_parameter("wvloT", [128, ND * HD], F8, isOutput=False)
    wo_h_e = nc.declare_dram_parameter("wohiT", [128, HPC * D], F8, isOutput=False)
    wo_l_e = nc.declare_dram_parameter("woloT", [128, HPC * D], F8, isOutput=False)
    cos_e = nc.declare_dram_parameter("cosT", [HD // 2, S], BF16, isOutput=False)
    sin_e = nc.declare_dram_parameter("sinT", [HD // 2, S], BF16, isOutput=False)
    yT_e = nc.declare_dram_parameter("yT", [D, B * S], BF16, isOutput=True)

    with tile.TileContext(nc) as tc:
        with (
            tc.tile_pool(name="const", bufs=1) as cpool,
            tc.tile_pool(name="w", bufs=1) as wpool,
            tc.tile_pool(name="x", bufs=1) as xpool,
            tc.tile_pool(name="act", bufs=1) as apool,
            tc.tile_pool(name="tmp", bufs=3) as tpool,
            tc.tile_pool(name="psA", bufs=4, space="PSUM") as psA,
            tc.tile_pool(name="psS", bufs=2, space="PSUM") as psS,
            tc.tile_pool(name="psB", bufs=2, space="PSUM") as psB,
        ):
            # ---- weight / table loads (nc.sync HWDGE), interleaved with the
            # first x tiles so the PE can start within ~4us.
            wk_h = wpool.tile([128, ND, HD], F8, tag="wkh", name="wkh")
            wk_l = wpool.tile([128, ND, HD], F8, tag="wkl", name="wkl")
            wq_h = wpool.tile([128, ND, HPC * HD], F8, tag="wqh", name="wqh")
            wq_l = wpool.tile([128, ND, HPC * HD], F8, tag="wql", name="wql")
            wv_h = wpool.tile([128, ND, HD], F8, tag="wvh", name="wvh")
            wv_l = wpool.tile([128, ND, HD], F8, tag="wvl", name="wvl")

            def ld_w(t, e, halves=1):
                # halves=2 splits the load so matmuls gated on the first
                # chunks can start after half the bytes land
                ea = e.rearrange("p (d o) -> p d o", d=ND)
                for i in range(halves):
                    sl = slice(i * ND // halves, (i + 1) * ND // halves)
                    nc.sync.dma_start(t[:, sl, :], ea[:, sl, :])

            xtiles = {}  # (b, hl, tb) -> [4 piece tiles]

            def ld_x(b, tb, hl):
                e = xh_e if hl == 0 else xl_e
                ts = []
                for pc in range(4):
                    t = xpool.tile(
                        [128, 4, 512], F8, tag=f"x{hl}{tb}{pc}",
                        name=f"x{hl}{tb}{pc}",
                    )
                    nc.sync.dma_start(
                        t[:],
                        e[
                            b,
                            pc * 512 : (pc + 1) * 512,
                            tb * 512 : (tb + 1) * 512,
                        ].rearrange("(d p) s -> p d s", p=128),
                    )
                    ts.append(t)
                xtiles[(b, hl, tb)] = ts

            # first k/q matmuls gate on these: thin, ordered loads. V runs as
            # a second pass so its weights load late; x tiles stream tb-major.
            # load order mirrors the A-first projection schedule: hi x tiles
            # and hi weights first, then the lo corrections, then tables/V/O.
            ld_w(wk_h, wk_h_e, halves=2)
            ld_x(0, 0, 0)
            ld_w(wq_h, wq_h_e, halves=2)
            ld_x(0, 1, 0)
            ld_x(0, 0, 1)
            ld_w(wk_l, wk_l_e)
            ld_w(wq_l, wq_l_e)
            cosT = cpool.tile([HD, S], BF16, tag="cos", name="cos")
            nc.sync.dma_start(cosT[0:64, :], cos_e[:, :])
            sinT = cpool.tile([HD, S], BF16, tag="sin", name="sin")
            nc.sync.dma_start(sinT[0:64, :], sin_e[:, :])
            nc.vector.tensor_copy(cosT[64:128, :], cosT[0:64, :])
            nc.vector.tensor_scalar_mul(sinT[64:128, :], sinT[0:64, :], -1.0)
            ld_x(0, 1, 1)
            ld_x(0, 2, 0)
            ld_x(0, 2, 1)
            ld_x(0, 3, 0)
            ld_x(0, 3, 1)
            ld_w(wv_h, wv_h_e)
            ld_w(wv_l, wv_l_e)
            wo_h = wpool.tile([128, HPC, D], F8, tag="woh", name="woh")
            nc.sync.dma_start(wo_h[:], wo_h_e.rearrange("p (c o) -> p c o", c=HPC))
            wo_l = wpool.tile([128, HPC, D], F8, tag="wol", name="wol")
            nc.sync.dma_start(wo_l[:], wo_l_e.rearrange("p (c o) -> p c o", c=HPC))

            ident = cpool.tile([128, 128], BF16, tag="ident", name="ident")
            make_identity(nc, ident[:])
            triu = cpool.tile([128, 128], BF16, tag="triu", name="triu")
            make_upper_triangular(nc, triu[:], val=1.0, diag=True)

            def rope(dst, acc, dsl, sl, scale, act_evac=True):
                # dsl: destination column slice in dst; sl: token slice for
                # the cos/sin tables (differ for the per-tb kTr tiles)
                ev = tpool.tile([128, 512], BF16, tag="ropee", name="ropee", bufs=2)
                if act_evac:
                    nc.scalar.mul(ev[:], acc[:], scale)
                else:
                    # DVE evac variant for fillers inside exp-bound windows
                    nc.vector.tensor_scalar_mul(ev[:], acc[:], scale)
                sw = tpool.tile([128, 512], BF16, tag="ropesw", name="ropesw", bufs=1)
                nc.vector.tensor_copy(sw[0:64, :], ev[64:128, :])
                nc.vector.tensor_copy(sw[64:128, :], ev[0:64, :])
                nc.vector.tensor_tensor(sw[:], sw[:], sinT[:, sl], op=MULT)
                nc.vector.tensor_tensor(ev[:], ev[:], cosT[:, sl], op=MULT)
                nc.vector.tensor_tensor(dst[:, dsl], ev[:], sw[:], op=ADD)

            # ---------- cross-batch pipelined schedule ----------
            # Batch b+1's k/q projection units run as PE fillers inside batch
            # b's attention loop (whose early strips are exp/ACT-bound), and
            # batch b's final oproj block runs as fillers in batch b+1's early
            # strips. qTr/kTr and x tiles are double-buffered to decouple the
            # batches; everything else reuses the same buffers (Tile's
            # region-granular deps keep it correct).
            yscale = 2.0 ** -(KW + KO)

            def dr_part(acc, w_h, w_l, csl, b, tb, part):
                xh = xtiles[(b, 0, tb)]
                xl = xtiles[(b, 1, tb)]
                ops = [(w_h, xh)] if part == 0 else [(w_h, xl), (w_l, xh)]
                n = 0 if part == 0 else ND // 2
                for wt, xt in ops:
                    for p in range(ND // 2):
                        lp = p % 2
                        nc.tensor.matmul(
                            acc[:],
                            wt[:, 2 * p : 2 * p + 2, csl],
                            xt[p // 2][:, 2 * lp : 2 * lp + 2, :],
                            start=(n == 0),
                            stop=(n == 3 * (ND // 2) - 1),
                            perf_mode=DR,
                        )
                        n += 1

            def alloc_qk():
                # q/k live in per-tb tiles (scores chunks are aligned to
                # absolute 512-col boundaries, so each chunk stays within one
                # tb tile). Single-buffered is then deadlock-free: the batch
                # b+1 rope for tb is emitted after every batch-b scores strip
                # that reads those columns (strips j < 4*tb+4), so its
                # whole-tile WAR wait only ever points backwards.
                qTr = [
                    [
                        apool.tile([HD, 512], BF16, tag=f"q{h}{tb}",
                                   name=f"q{h}{tb}")
                        for tb in range(NT)
                    ]
                    for h in range(HPC)
                ]
                kTr = [
                    apool.tile([HD, 512], BF16, tag=f"k{tb}", name=f"k{tb}")
                    for tb in range(NT)
                ]
                return qTr, kTr

            def kqproj_units(qTr, kTr):
                units = []
                for tb in range(NT):
                    units.append(
                        (wk_h, wk_l, slice(0, HD), tb, kTr[tb],
                         slice(0, 512), 2.0**-KK)
                    )
                    for h in range(HPC):
                        units.append(
                            (wq_h, wq_l, slice(h * HD, (h + 1) * HD), tb,
                             qTr[h][tb], slice(0, 512), 2.0**-KQ)
                        )
                return units

            def emit_unit(u, b, act_evac=True):
                w_h, w_l, csl, tb, dst, dsl, sc = u
                acc = psA.tile([128, 512], mybir.dt.float32, tag="acc",
                               name="acc")
                dr_part(acc, w_h, w_l, csl, b, tb, 0)
                dr_part(acc, w_h, w_l, csl, b, tb, 1)
                rope(dst, acc, dsl, slice(tb * 512, (tb + 1) * 512), sc,
                     act_evac)

            def kqproj_startup(b, qTr, kTr):
                # Startup k/q projection: per tb-pair, the six hi*hi product
                # chains first (they only need the hi x/w tiles, which arrive
                # first), then the hi*lo / lo*hi corrections + evacuations.
                # Tracks the DMA arrival order so the PE starts within ~4us.
                all_units = kqproj_units(qTr, kTr)
                for tbp in range(NT // 2):
                    units = [
                        all_units[i]
                        for i in range(3 * 2 * tbp, 3 * 2 * tbp + 6)
                    ]
                    if tbp == NT // 2 - 1:
                        # q ropes first on the final pair: the first scores
                        # strips consume qTr, k chunk 0 is already resident
                        units = units[1:3] + [units[0]] + units[4:6] + [units[3]]
                    accs = []
                    for w_h, w_l, csl, tb, dst, dsl, sc in units:
                        acc = psA.tile(
                            [128, 512], mybir.dt.float32, tag="acc", name="acc"
                        )
                        dr_part(acc, w_h, w_l, csl, b, tb, 0)
                        accs.append(acc)
                    for acc, (w_h, w_l, csl, tb, dst, dsl, sc) in zip(
                        accs, units
                    ):
                        dr_part(acc, w_h, w_l, csl, b, tb, 1)
                        rope(dst, acc, dsl, slice(tb * 512, (tb + 1) * 512),
                             sc)

            def oproj_quad(b, oth, otl, cg, q, act_ok=True, sync_dma=False):
                # Four dc-blocks in two [128, 2, 512] staging tiles + two
                # strided DMAs. Quads are spread one-per-strip so the PSUM
                # ring never sees a 16-tile demand burst. act_ok=False keeps
                # every evac on DVE (for exp-saturated windows).
                tsl = slice(cg * 512, (cg + 1) * 512)
                for half in range(2):
                    yrq = tpool.tile([128, 2, 512], BF16, tag="yrq",
                                     name="yrq", bufs=3)
                    for i in range(2):
                        dc = q * 4 + half * 2 + i
                        dsl = slice(dc * 128, (dc + 1) * 128)
                        yps = psA.tile([128, 512], mybir.dt.float32,
                                       tag="acc", name="yps")
                        for n, (wt, ot) in enumerate(
                            ((wo_h, oth), (wo_h, otl), (wo_l, oth))
                        ):
                            nc.tensor.matmul(
                                yps[:],
                                wt[:, :, dsl],
                                ot[:, :, tsl],
                                start=(n == 0),
                                stop=(n == 2),
                                perf_mode=DR,
                            )
                        if act_ok and half * 2 + i == 1:
                            nc.scalar.mul(yrq[:, i, :], yps[:], yscale)
                        else:
                            nc.vector.tensor_scalar_mul(yrq[:, i, :], yps[:],
                                                        yscale)
                    eng = nc.sync if sync_dma else nc.gpsimd
                    eng.dma_start(
                        yT_e[q * 512 + half * 256 : q * 512 + half * 256 + 256,
                             b * S + cg * 512 : b * S + (cg + 1) * 512]
                        .rearrange("(d p) s -> p d s", p=128),
                        yrq[:],
                    )

            def oproj_tail(b, oth, otl, c0, groups):
                # Fine-grained tail for the last batch's final token blocks:
                # a 256-wide column slab, split into quarter-height DMAs
                # (256-col bf16 rows = 512B descriptors: still full DMA rate)
                # so the staging tag stays a small double buffer.
                tsl = slice(c0, c0 + 256)
                for g in groups:
                    # staging reuses the yrq tag: [128, 2, 512] has the same
                    # per-partition linear layout as the [128, 4, 256] DMA view
                    yt = tpool.tile([128, 2, 512], BF16, tag="yrq",
                                    name="yrq", bufs=3)
                    for i in range(4):
                        dc = g * 4 + i
                        dsl = slice(dc * 128, (dc + 1) * 128)
                        ysl = slice((i % 2) * 256, (i % 2) * 256 + 256)
                        yps = psA.tile([128, 512], mybir.dt.float32,
                                       tag="acc", name="yps")
                        for n, (wt, ot) in enumerate(
                            ((wo_h, oth), (wo_h, otl), (wo_l, oth))
                        ):
                            nc.tensor.matmul(
                                yps[:, 0:256],
                                wt[:, :, dsl],
                                ot[:, :, tsl],
                                start=(n == 0),
                                stop=(n == 2),
                                perf_mode=DR,
                            )
                        if i % 2 == 1:
                            nc.scalar.mul(yt[:, i // 2, ysl], yps[:, 0:256],
                                          yscale)
                        else:
                            nc.vector.tensor_scalar_mul(
                                yt[:, i // 2, ysl], yps[:, 0:256], yscale
                            )
                    # HWDGE: the sync queue is idle by the tail, and its
                    # per-DMA latency is far below the engine SWDGE path —
                    # these DMAs gate the final drain
                    nc.sync.dma_start(
                        yT_e[g * 512 : (g + 1) * 512,
                             b * S + c0 : b * S + c0 + 256]
                        .rearrange("(d p) s -> p d s", p=128),
                        yt[:],
                    )

            def attention(b, qTr, kTr, fillers):
                """Attention for batch b; `fillers` is a deque of (slot, fn)
                PE-filler closures emitted once the loop index reaches slot.
                Returns filler closures for batch b+1 (this batch's final
                oproj block), or [] for the last batch (handled via the
                fine-grained tail)."""
                vnat = [
                    apool.tile([128, HD + 1], BF16, tag=f"vn{j}", name=f"vn{j}")
                    for j in range(NI)
                ]

                def vproj(j):
                    # V natural: x stationary, wv moving, per 128-wide
                    # t-chunk. Runs inside the attention loop (chunk j is
                    # first needed by pv(c=j) one block later) to fill the PE
                    # during the exp-heavy early strips.
                    tb, i = divmod(j, 4)
                    vacc = psA.tile(
                        [128, HD], mybir.dt.float32, tag="acc", name="vacc"
                    )
                    xh = xtiles[(b, 0, tb)]
                    xl = xtiles[(b, 1, tb)]
                    tsl = slice(i * 128, (i + 1) * 128)
                    ops = [(xh, wv_h), (xl, wv_h), (xh, wv_l)]
                    n = 0
                    for xt, wt in ops:
                        for p in range(ND // 2):
                            lp = p % 2
                            nc.tensor.matmul(
                                vacc[:],
                                xt[p // 2][:, 2 * lp : 2 * lp + 2, tsl],
                                wt[:, 2 * p : 2 * p + 2, :],
                                start=(n == 0),
                                stop=(n == 3 * (ND // 2) - 1),
                                perf_mode=DR,
                            )
                            n += 1
                    # DVE evac: keeps ACT free for the exp stream. The ones
                    # column is pre-scaled by 2^-KO so the PV-normalize
                    # reciprocal comes out as 2^KO/denom (the fp8 O^T
                    # pre-scale), letting the normalize run on ACT as a
                    # single scaled copy.
                    nc.vector.tensor_scalar_mul(vnat[j][:, 0:HD], vacc[:],
                                                2.0**-KK)
                    nc.gpsimd.memset(vnat[j][:, HD : HD + 1], 2.0**-KO)

                strips = {h: [] for h in range(HPC)}
                for h in range(HPC):
                    for j in range(NI):
                        strips[h].append(
                            apool.tile(
                                [128, S - j * 128],
                                BF16,
                                tag=f"pt{h}{j}",
                                name=f"pt{h}{j}",
                            )
                        )
                # O^T in fp8 hi+lo (oproj runs as 3-product DoubleRow):
                # [:, h, :] holds head h's 128 dims, so [:, :, tsl] is the
                # natural DR moving layout pairing both heads' chunks.
                # Double-buffered: batch b's oproj quads spread as fillers
                # through batch b+1's attention window.
                oth = apool.tile([128, HPC, S], F8, tag="oth", name="oth",
                                 bufs=2)
                otl = apool.tile([128, HPC, S], F8, tag="otl", name="otl",
                                 bufs=2)

                def scores(j, h):
                    # chunks aligned to absolute 512-col boundaries: same
                    # chunk count as naive splitting, but each chunk reads a
                    # single per-tb qTr tile
                    pts = strips[h][j]
                    a = j * 128
                    while a < S:
                        w = min(512 - a % 512, S - a)
                        sps = psS.tile([128, 512], mybir.dt.float32,
                                       tag="sps", name="s")
                        nc.tensor.matmul(
                            sps[:, :w],
                            kTr[j // 4][:, (j % 4) * 128 : (j % 4 + 1) * 128],
                            qTr[h][a // 512][:, a % 512 : a % 512 + w],
                            start=True,
                            stop=True,
                        )
                        c0 = a - j * 128
                        nc.scalar.activation(pts[:, c0 : c0 + w], sps[:, :w],
                                             EXP)
                        if c0 == 0:
                            # causal mask on Pool: DVE is the loaded engine
                            # in the attention windows, Pool is nearly idle
                            nc.gpsimd.tensor_tensor(
                                pts[:, 0:128], pts[:, 0:128], triu[:], op=MULT
                            )
                        a += w

                # pv stage 1: PV matmuls + kick the DVE normalize; stage 2
                # (one strip later): PE transpose + split into fp8 O^T hi/lo,
                # so the PE never waits on the DVE chain.
                pv_osb = {}

                def pv_mm(c, h):
                    oext = psA.tile([128, 512], mybir.dt.float32, tag="acc",
                                    name="oext")
                    for j in range(c + 1):
                        nc.tensor.matmul(
                            oext[:, 0 : HD + 1],
                            strips[h][j][:, (c - j) * 128 : (c - j + 1) * 128],
                            vnat[j][:],
                            start=(j == 0),
                            stop=(j == c),
                        )
                    osb = tpool.tile([128, HD], BF16, tag=f"onat{h}",
                                     name="onat", bufs=2)
                    rcol = tpool.tile([128, 1], mybir.dt.float32, tag=f"rc{h}",
                                      name="rcol", bufs=2)
                    nc.vector.reciprocal(rcol[:], oext[:, HD : HD + 1])
                    # normalize (rcol = 2^KO/denom via the scaled ones
                    # column) as a scaled copy on ACT for the mid strips:
                    # keeps the DVE queue, which gates the downstream
                    # transposes, short. Early strips (ACT exp-bound) and the
                    # last strips (ACT gates the tail evacs) stay on DVE.
                    if 6 <= c:
                        nc.scalar.activation(
                            osb[:], oext[:, 0:HD],
                            mybir.ActivationFunctionType.Copy, scale=rcol[:]
                        )
                    else:
                        nc.vector.tensor_scalar(
                            osb[:], oext[:, 0:HD], rcol[:], None, op0=MULT
                        )
                    pv_osb[(c, h)] = osb

                def pv_tp(c, h):
                    tp = psB.tile([128, 128], BF16, tag="pt", name="pt")
                    nc.tensor.transpose(tp[:], pv_osb.pop((c, h))[:], ident[:])
                    csl = slice(c * 128, (c + 1) * 128)
                    nc.vector.tensor_copy(oth[:, h, csl], tp[:])
                    nc.vector.tensor_tensor(
                        otl[:, h, csl], tp[:], oth[:, h, csl],
                        op=mybir.AluOpType.subtract,
                    )

                # tb0's four v-chunks up front: this batch's x tb0 is then
                # dead, so the next batch's tb0 load (WAR-ordered) starts at
                # the top of this window. The rest run two-per-strip — the
                # early strips are exp-bound and need the PE filler, and x
                # tbs die sooner, pulling the next batch's loads forward.
                for jj in range(4):
                    vproj(jj)
                if b + 1 < B:
                    ld_x(b + 1, 0, 0)
                    ld_x(b + 1, 0, 1)
                last = b + 1 >= B
                quads = []
                vnext = 4
                for j in range(NI + 2):
                    if j < NI:
                        for h in range(HPC):
                            scores(j, h)
                    for _ in range(2):
                        if vnext < NI:
                            vproj(vnext)
                            if vnext % 4 == 3 and b + 1 < B:
                                ld_x(b + 1, vnext // 4, 0)
                                ld_x(b + 1, vnext // 4, 1)
                            vnext += 1
                    if 1 <= j <= NI:
                        for h in range(HPC):
                            pv_mm(j - 1, h)
                    if j >= 2:
                        c2 = j - 2
                        for h in range(HPC):
                            pv_tp(c2, h)
                        if c2 % 4 == 3 and c2 // 4 < NT - 1:
                            quads.extend((c2 // 4, q) for q in range(4))
                        if quads:
                            # the last batch's window is exp-saturated with
                            # no next-batch fillers: keep its quad evacs on
                            # DVE
                            # the last batch's window has no next-batch x
                            # loads: its quad DMAs ride the idle HWDGE queue
                            oproj_quad(b, oth, otl, *quads.pop(0),
                                       act_ok=True, sync_dma=last)
                        if last:
                            # fine-grained tail for the final 512 tokens as
                            # two strip-pair slabs, so only the last ~1MB of
                            # output DMA trails the final transposes
                            if c2 == NI - 3:
                                oproj_tail(b, oth, otl, S - 512, range(0, 2))
                            elif c2 == NI - 2:
                                oproj_tail(b, oth, otl, S - 512, range(2, 4))
                            elif c2 == NI - 1:
                                oproj_tail(b, oth, otl, S - 256, range(4))
                    while fillers and fillers[0][0] <= j:
                        fillers.popleft()[1]()
                while fillers:
                    fillers.popleft()[1]()
                if last:
                    return []
                # the final block cg=3 goes to the next batch's early window
                # (oth/otl are double-buffered, so these reads never collide
                # with batch b+1's pv_tp writes)
                cg = NT - 1
                return [
                    (q, partial(oproj_quad, b, oth, otl, cg, q, False, True))
                    for q in range(4)
                ]

            qk = [None] * B
            qk[0] = alloc_qk()
            kqproj_startup(0, *qk[0])
            fillers = deque()
            for b in range(B):
                if b + 1 < B:
                    qk[b + 1] = alloc_qk()
                    units = kqproj_units(*qk[b + 1])
                    for i, u in enumerate(units):
                        # one unit per strip from slot 3: with the 2x vproj
                        # pacing, x tb lands ~2.5 strips before its units;
                        # early slots evacuate the rope via DVE: ACT is
                        # exp-saturated in the wide strips
                        fillers.append(
                            (3 + i, partial(emit_unit, u, b + 1, i >= 4))
                        )
                nxt = attention(b, *qk[b], fillers)
                fillers = deque(nxt)

    _split_multi_waits(nc)
    nc.finalize()
    return nc


def _get_nc():
    if "nc" not in _BUILD_CACHE:
        _BUILD_CACHE["nc"] = _build()
    return _BUILD_CACHE["nc"]


def _prep_inputs(x, wq, wk, wv, wo):
    """Host-side shard prep: per-core transposed fp8 hi/lo weight splits and
    shared fp8 hi/lo x^T."""
    bf16 = ml_dtypes.bfloat16
    f8 = ml_dtypes.float8_e4m3

    xT = np.ascontiguousarray(np.transpose(x, (0, 2, 1)))
    xhi = xT.astype(f8)
    xlo = (xT - xhi.astype(np.float32)).astype(f8)

    # RoPE tables in [hd, s] layout; emb = concat([ang, ang]).
    inv_freq = 1.0 / (THETA ** (np.arange(0, HD, 2, dtype=np.float32) / HD))
    ang = np.arange(S, dtype=np.float32)[:, None] * inv_freq[None, :]  # [S, HD/2]
    cosT = np.cos(ang).T.astype(bf16)  # [HD/2, S]; device mirrors to 64..127
    sinT = (-np.sin(ang).T).astype(bf16)  # negated rows 0..63; device flips sign

    scale = 1.0 / math.sqrt(HD)

    def split(a, k):
        # a: [D, M] transposed weight; returns fp8 hi/lo pre-swizzled into
        # the device SBUF layout [128, ND*M] (partition p holds row d*128+p)
        s = np.ascontiguousarray(a * 2.0**k).astype(np.float32)
        hi = s.astype(f8)
        lo = (s - hi.astype(np.float32)).astype(f8)

        def swz(w):
            m = w.shape[1]
            return np.ascontiguousarray(
                w.reshape(ND, 128, m).transpose(1, 0, 2).reshape(128, ND * m)
            )

        return swz(hi), swz(lo)

    in_maps = []
    for c in range(NCORES):
        h0 = c * HPC
        g = (c * HPC) // (H // KV)
        wq_hi, wq_lo = split((wq[h0 * HD : (h0 + HPC) * HD, :] * scale).T, KQ)
        wk_hi, wk_lo = split(wk[g * HD : (g + 1) * HD, :].T, KK)
        wv_hi, wv_lo = split(wv[g * HD : (g + 1) * HD, :].T, KK)
        # wo fp8 hi/lo in [128, HPC*D] (partition p, chunk c -> row c*128+p)
        wo_c = wo[:, h0 * HD : (h0 + HPC) * HD].T  # [HPC*HD, D]
        ws = np.ascontiguousarray(wo_c * 2.0**KW).astype(np.float32)
        wo_hi8 = ws.astype(f8)
        wo_lo8 = (ws - wo_hi8.astype(np.float32)).astype(f8)

        def swz_o(w):
            return np.ascontiguousarray(
                w.reshape(HPC, 128, D).transpose(1, 0, 2).reshape(128, HPC * D)
            )
        in_maps.append(
            {
                "xhiT": xhi,
                "xloT": xlo,
                "wqhiT": wq_hi,
                "wqloT": wq_lo,
                "wkhiT": wk_hi,
                "wkloT": wk_lo,
                "wvhiT": wv_hi,
                "wvloT": wv_lo,
                "wohiT": swz_o(wo_hi8),
                "woloT": swz_o(wo_lo8),
                "cosT": cosT,
                "sinT": sinT,
            }
        )
    return in_maps


def _gather(results):
    acc = np.zeros((D, B * S), np.float32)
    for r in results:
        acc += r["yT"].astype(np.float32)
    return np.ascontiguousarray(acc.T).reshape(B, S, D)


def kernel(x, wq, wk, wv, wo):
    from concourse.bass_utils import run_bass_kernel_spmd

    # Coerce to host numpy: device-array inputs must not trigger on-device
    # host math in _prep_inputs.
    x = np.asarray(x, dtype=np.float32)
    wq = np.asarray(wq, dtype=np.float32)
    wk = np.asarray(wk, dtype=np.float32)
    wv = np.asarray(wv, dtype=np.float32)
    wo = np.asarray(wo, dtype=np.float32)

    nc = _get_nc()
    in_maps = _prep_inputs(x, wq, wk, wv, wo)
    res = run_bass_kernel_spmd(nc, in_maps, core_ids=list(range(NCORES)))
    return _gather(res.results)

